# revision 7
# baseline (speedup 1.0000x reference)
"""Trainium2 Bass kernel for CausalSelfAttention (RoPE + ALiBi + causal mask).

Sharding: 16 heads tensor-parallel across 8 NeuronCores (2 heads/core).
Per core:
  phase 1: qkv projection from replicated x^T; RoPE applied on the fly.
           q^T,k^T kept in SBUF [d, t]; v stored to DRAM in natural [t, d].
  phase 2: attention per (batch, head) in transposed layout
           S^T[j, i] = k^T.T @ q^T; ALiBi+mask via precomputed additive
           [128,128] delta tiles; exp on ScalarE; row-sums via ones-matmul;
           y^T accumulated on TensorE; normalized by broadcast reciprocal.
  phase 3: out partial = y @ W_proj (rows of the core's heads).
Host: sums the 8 partial outputs.

All matmuls run in float32r (TF32-like, full PE rate at free dim >= 256).
"""

import math
from contextlib import ExitStack

import numpy as np

import concourse.bass as bass
import concourse.mybir as mybir
import concourse.tile as tile
from concourse import bacc
from concourse.bass_utils import run_bass_kernel_spmd

B, T, DM = 2, 2048, 2048
H, HD = 16, 128
ROWS = B * T                      # 4096
NCORES = 8
HPC = H // NCORES                 # 2 heads per core
ROPE_THETA = 10000.0
SQHD = math.sqrt(HD)
M_OFF = 18.0                      # softmax stability offset
NEG = -1.0e30

TCH = 512                         # t-chunk width in phase 1
NCH = ROWS // TCH                 # 8
CT = DM // 128                    # 16 contraction tiles
NT = T // 128                     # 16 key/query tiles per batch
IC = 512                          # query chunk in phase 2
NIC = T // IC                     # 4

F32 = mybir.dt.float32
F32R = mybir.dt.float32r
MULT = mybir.AluOpType.mult
ADD = mybir.AluOpType.add
EXP = mybir.ActivationFunctionType.Exp


def build_program():
    nc = bacc.Bacc("TRN2", target_bir_lowering=False, debug=False,
                   num_devices=NCORES)
    xT = nc.dram_tensor("xT", [DM, ROWS], F32, kind="ExternalInput").ap()
    wq = nc.dram_tensor("wq", [DM, HPC * HD], F32, kind="ExternalInput").ap()
    wk = nc.dram_tensor("wk", [DM, HPC * HD], F32, kind="ExternalInput").ap()
    wv = nc.dram_tensor("wv", [DM, HPC * HD], F32, kind="ExternalInput").ap()
    wp = nc.dram_tensor("wp", [HPC * HD, DM], F32, kind="ExternalInput").ap()
    cosT = nc.dram_tensor("cosT", [128, T], F32, kind="ExternalInput").ap()
    sinT = nc.dram_tensor("sinT", [128, T], F32, kind="ExternalInput").ap()
    prot = nc.dram_tensor("prot", [128, 128], F32, kind="ExternalInput").ap()
    biasd = nc.dram_tensor("biasd", [128, HPC, 19, 128], F32,
                           kind="ExternalInput").ap()
    ones128 = nc.dram_tensor("ones128", [128, 1], F32, kind="ExternalInput").ap()
    ones1 = nc.dram_tensor("ones1", [1, 128], F32, kind="ExternalInput").ap()
    out = nc.dram_tensor("out", [ROWS, DM], F32, kind="ExternalOutput").ap()

    xT3 = xT.rearrange("(o p) t -> p o t", p=128)

    with tile.TileContext(nc) as tc, ExitStack() as ctx:
        const = ctx.enter_context(tc.tile_pool(name="const", bufs=1))
        dram = ctx.enter_context(tc.tile_pool(name="dram", bufs=1, space="DRAM"))
        qkp = ctx.enter_context(tc.tile_pool(name="qk", bufs=1))

        q_sb = [qkp.tile([128, ROWS], F32R, tag=f"q{e}", name=f"q{e}")
                for e in range(HPC)]
        k_sb = [qkp.tile([128, ROWS], F32R, tag=f"k{e}", name=f"k{e}")
                for e in range(HPC)]
        v_dram = dram.tile([ROWS, HPC * HD], F32)

        cos_sb = const.tile([128, T], F32, tag="cos")
        sin_sb = const.tile([128, T], F32, tag="sin")
        prot_sb = const.tile([128, 128], F32R, tag="prot")
        ones128_sb = const.tile([128, 1], F32R, tag="o128")
        ones1_sb = const.tile([1, 128], F32R, tag="o1")
        nc.sync.dma_start(cos_sb[:], cosT)
        nc.sync.dma_start(sin_sb[:], sinT)
        nc.sync.dma_start(prot_sb[:], prot.bitcast(F32R))
        nc.sync.dma_start(ones128_sb[:], ones128.bitcast(F32R))
        nc.sync.dma_start(ones1_sb[:], ones1.bitcast(F32R))

        # ---------------- phase 1: qkv + rope ----------------
        with tc.tile_pool(name="w1", bufs=1) as wpool, \
             tc.tile_pool(name="xt", bufs=20) as xpool, \
             tc.tile_pool(name="rope", bufs=3) as rpool, \
             tc.tile_pool(name="vst", bufs=3) as vstp, \
             tc.tile_pool(name="p1", bufs=3, space="PSUM") as ps1, \
             tc.tile_pool(name="pv", bufs=2, space="PSUM") as psv:
            wq_sb = wpool.tile([128, CT, HPC * HD], F32R, tag="wq")
            wk_sb = wpool.tile([128, CT, HPC * HD], F32R, tag="wk")
            wv_sb = wpool.tile([128, CT, HPC * HD], F32R, tag="wv")
            nc.sync.dma_start(wq_sb[:], wq.rearrange("(o p) e -> p o e", p=128).bitcast(F32R))
            nc.sync.dma_start(wk_sb[:], wk.rearrange("(o p) e -> p o e", p=128).bitcast(F32R))
            nc.sync.dma_start(wv_sb[:], wv.rearrange("(o p) e -> p o e", p=128).bitcast(F32R))

            for tchunk in range(NCH):
                t0 = tchunk * TCH
                xts = []
                for ci in range(CT):
                    xt_t = xpool.tile([128, TCH], F32R, tag="xt")
                    nc.sync.dma_start(xt_t[:], xT3[:, ci, t0:t0 + TCH].bitcast(F32R))
                    xts.append(xt_t)

                cs = slice(t0 % T, t0 % T + TCH)
                for dst, w_sb in ((q_sb, wq_sb), (k_sb, wk_sb)):
                    for et in range(HPC):
                        ps_q = ps1.tile([128, TCH], F32, tag="psq")
                        for ci in range(CT):
                            nc.tensor.matmul(ps_q[:],
                                             w_sb[:, ci, et * HD:(et + 1) * HD],
                                             xts[ci][:],
                                             start=(ci == 0), stop=(ci == CT - 1))
                        qraw = rpool.tile([128, TCH], F32R, tag="qraw")
                        nc.scalar.copy(qraw[:], ps_q[:])
                        ps_r = ps1.tile([128, TCH], F32, tag="psrot")
                        nc.tensor.matmul(ps_r[:], prot_sb[:], qraw[:],
                                         start=True, stop=True)
                        tmp = rpool.tile([128, TCH], F32, tag="tmp")
                        nc.vector.tensor_mul(tmp[:], ps_r[:], sin_sb[:, cs])
                        dcols = dst[et][:, t0:t0 + TCH]
                        nc.vector.tensor_mul(dcols, qraw[:], cos_sb[:, cs])
                        nc.vector.tensor_add(dcols, dcols, tmp[:])

                for tt in range(TCH // 128):
                    ps_vt = psv.tile([128, HPC * HD], F32, tag="psv")
                    for ci in range(CT):
                        nc.tensor.matmul(ps_vt[:],
                                         xts[ci][:, tt * 128:(tt + 1) * 128],
                                         wv_sb[:, ci, :],
                                         start=(ci == 0), stop=(ci == CT - 1))
                    v_stage = vstp.tile([128, HPC * HD], F32, tag="vst")
                    nc.scalar.copy(v_stage[:], ps_vt[:])
                    r0 = t0 + tt * 128
                    nc.sync.dma_start(v_dram[r0:r0 + 128, :], v_stage[:])

        # ---------------- phase 2+3: attention + projection ----------------
        with tc.tile_pool(name="c2", bufs=1) as const2, \
             tc.tile_pool(name="wt", bufs=6) as wpool2, \
             tc.tile_pool(name="vt", bufs=2) as vpool, \
             tc.tile_pool(name="yb", bufs=2) as ypool, \
             tc.tile_pool(name="sm", bufs=4) as smpool, \
             tc.tile_pool(name="ost", bufs=4) as ostp, \
             tc.tile_pool(name="pssc", bufs=2, space="PSUM") as pssc, \
             tc.tile_pool(name="psacc", bufs=2, space="PSUM") as psacc, \
             tc.tile_pool(name="psm", bufs=1, space="PSUM") as psmisc, \
             tc.tile_pool(name="pso", bufs=2, space="PSUM") as pso:
            bias_sb = const2.tile([128, HPC, 19, 128], F32, tag="bias")
            nc.sync.dma_start(bias_sb[:], biasd)
            wp_sb = const2.tile([128, HPC, DM], F32R, tag="wp")
            nc.sync.dma_start(wp_sb[:], wp.rearrange("(o p) e -> p o e", p=128).bitcast(F32R))

            for b in range(B):
                y_b = ypool.tile([128, HPC, T], F32R, tag="yb")
                for hi in range(HPC):
                    vt = vpool.tile([128, NT, HD], F32R, tag="vt")
                    nc.sync.dma_start(
                        vt[:],
                        v_dram[b * T:(b + 1) * T, hi * HD:(hi + 1) * HD]
                        .rearrange("(o p) e -> p o e", p=128).bitcast(F32R))
                    for icx in range(NIC):
                        i0 = b * T + icx * IC
                        jt_hi = (icx + 1) * (IC // 128)
                        ps_y = psacc.tile([128, IC], F32, tag="psy")
                        ps_sum = psmisc.tile([1, IC], F32, tag="pssum")
                        for jt in range(jt_hi):
                            ps_sc = pssc.tile([128, IC], F32, tag="pssc")
                            nc.tensor.matmul(
                                ps_sc[:],
                                k_sb[hi][:, b * T + jt * 128: b * T + (jt + 1) * 128],
                                q_sb[hi][:, i0:i0 + IC],
                                start=True, stop=True)
                            d0 = icx * (IC // 128) - jt
                            ps3 = ps_sc[:].rearrange("p (a c) -> p a c", c=128)
                            nc.vector.scalar_tensor_tensor(
                                out=ps3, in0=ps3, scalar=1.0,
                                in1=bias_sb[:, hi, d0 + 3:d0 + 7, :],
                                op0=MULT, op1=ADD)
                            w_t = wpool2.tile([128, IC], F32R, tag="wt")
                            nc.scalar.activation(w_t[:], ps_sc[:], EXP,
                                                 bias=0.0, scale=1.0 / SQHD)
                            nc.tensor.matmul(ps_y[:], vt[:, jt, :], w_t[:],
                                             start=(jt == 0), stop=(jt == jt_hi - 1))
                            nc.tensor.matmul(ps_sum[:], ones128_sb[:], w_t[:],
                                             start=(jt == 0), stop=(jt == jt_hi - 1))
                        recip = smpool.tile([1, IC], F32R, tag="recip")
                        with nc.allow_low_precision(reason="f32r is 4-byte"):
                            nc.vector.reciprocal(recip[:], ps_sum[:])
                        ps_b = psmisc.tile([128, IC], F32, tag="psb")
                        nc.tensor.matmul(ps_b[:], ones1_sb[:], recip[:],
                                         start=True, stop=True)
                        bca = smpool.tile([128, IC], F32, tag="bca")
                        nc.scalar.copy(bca[:], ps_b[:])
                        nc.vector.tensor_mul(y_b[:, hi, icx * IC:(icx + 1) * IC],
                                             ps_y[:], bca[:])

                for tt in range(NT):
                    for ec in range(DM // 512):
                        ps_out = pso.tile([128, 512], F32, tag="pso")
                        for dt_ in range(HPC):
                            nc.tensor.matmul(ps_out[:],
                                             y_b[:, dt_, tt * 128:(tt + 1) * 128],
                                             wp_sb[:, dt_, ec * 512:(ec + 1) * 512],
                                             start=(dt_ == 0), stop=(dt_ == HPC - 1))
                        o_stage = ostp.tile([128, 512], F32, tag="ost")
                        nc.any.tensor_copy(o_stage[:], ps_out[:])
                        r0 = b * T + tt * 128
                        nc.sync.dma_start(out[r0:r0 + 128, ec * 512:(ec + 1) * 512],
                                          o_stage[:])

    nc.compile()
    return nc


def _host_tensors():
    """Core-independent constant inputs."""
    hd2 = HD // 2
    inv_freq = 1.0 / (ROPE_THETA ** (np.arange(0, HD, 2, dtype=np.float64) / HD))
    ang = np.arange(T, dtype=np.float64)[:, None] * inv_freq[None, :]   # [T, 64]
    cos_h = np.cos(ang).T.astype(np.float32)                            # [64, T]
    sin_h = np.sin(ang).T.astype(np.float32)
    cosT = np.concatenate([cos_h, cos_h], axis=0)                       # [128, T]
    sinT = np.concatenate([sin_h, sin_h], axis=0)

    prot = np.zeros((128, 128), dtype=np.float32)
    for e in range(hd2):
        prot[e + hd2, e] = -1.0       # rot_e = -q_{e+64}  (e < 64)
        prot[e, e + hd2] = 1.0        # rot_e = +q_{e-64}  (e >= 64)

    ones128 = np.ones((128, 1), dtype=np.float32)
    ones1 = np.ones((1, 128), dtype=np.float32)
    return cosT, sinT, prot, ones128, ones1


def _bias_tiles(h0):
    """[128, HPC, 19, 128] additive pre-scale bias: sqrt(HD)*(alibi - M) or NEG."""
    jj = np.arange(128)[:, None]
    ii = np.arange(128)[None, :]
    rel = (jj - ii).astype(np.float64)          # (jj - ii)
    bias = np.empty((128, HPC, 19, 128), dtype=np.float32)
    for e in range(HPC):
        h = h0 + e
        slope = 2.0 ** (-8.0 * (h + 1) / H)
        for di in range(19):
            d = di - 3                           # d = it - jt
            if d < 0:
                tile_v = np.full((128, 128), NEG, dtype=np.float32)
            else:
                v = SQHD * (slope * (rel - 128.0 * d) - M_OFF)
                tile_v = v.astype(np.float32)
                if d == 0:
                    tile_v = np.where(jj > ii, NEG, tile_v)
            bias[:, e, di, :] = tile_v
    return bias


_NC_CACHE = {}


def _get_program():
    if "nc" not in _NC_CACHE:
        _NC_CACHE["nc"] = build_program()
    return _NC_CACHE["nc"]


def make_in_maps(x, W_qkv, W_proj):
    x = np.asarray(x, dtype=np.float32)
    W_qkv = np.asarray(W_qkv, dtype=np.float32)
    W_proj = np.asarray(W_proj, dtype=np.float32)

    xT = np.ascontiguousarray(x.reshape(ROWS, DM).T)        # [DM, ROWS]
    Wq, Wk, Wv = W_qkv[:, :DM], W_qkv[:, DM:2 * DM], W_qkv[:, 2 * DM:]
    cosT, sinT, prot, ones128, ones1 = _host_tensors()

    in_maps = []
    for c in range(NCORES):
        h0 = HPC * c
        cols = np.r_[h0 * HD:(h0 + 1) * HD, (h0 + 1) * HD:(h0 + 2) * HD]
        in_maps.append({
            "xT": xT,
            "wq": np.ascontiguousarray(Wq[:, cols]),
            "wk": np.ascontiguousarray(Wk[:, cols]),
            "wv": np.ascontiguousarray(Wv[:, cols]),
            "wp": np.ascontiguousarray(W_proj[cols, :]),
            "cosT": cosT,
            "sinT": sinT,
            "prot": prot,
            "biasd": _bias_tiles(h0),
            "ones128": ones128,
            "ones1": ones1,
        })
    return in_maps


def kernel(x, causal_mask, W_qkv, W_proj):
    del causal_mask  # always lower-triangular; causality is hardcoded
    nc = _get_program()
    in_maps = make_in_maps(x, W_qkv, W_proj)
    res = run_bass_kernel_spmd(nc, in_maps, core_ids=list(range(NCORES)))
    acc = np.zeros((ROWS, DM), dtype=np.float32)
    for c in range(NCORES):
        acc += res.results[c]["out"]
    return acc.reshape(B, T, DM)


# revision 18
# speedup vs baseline: 4.5766x; 4.5766x over previous
"""Trainium2 Bass kernel for CausalSelfAttention (RoPE + ALiBi + causal mask).

Sharding: 16 heads tensor-parallel across 8 NeuronCores (2 heads/core).
Per core:
  phase 1: qkv projection from replicated x^T; RoPE applied on the fly.
           q^T,k^T kept in SBUF [d, t]; v stored to DRAM in natural [t, d].
  phase 2: attention per (batch, head) in transposed layout
           S^T[j, i] = k^T.T @ q^T; ALiBi+mask via precomputed additive
           [128,128] delta tiles; exp on ScalarE; row-sums via ones-matmul;
           y^T accumulated on TensorE; normalized by broadcast reciprocal.
  phase 3: out partial = y @ W_proj (rows of the core's heads).
Host: sums the 8 partial outputs.

All matmuls run in float32r (TF32-like, full PE rate at free dim >= 256).
"""

import math
from contextlib import ExitStack

import numpy as np

import concourse.bass as bass
import concourse.mybir as mybir
import concourse.tile as tile
from concourse import bacc
from concourse.bass_utils import run_bass_kernel_spmd

B, T, DM = 2, 2048, 2048
H, HD = 16, 128
ROWS = B * T                      # 4096
NCORES = 8
HPC = H // NCORES                 # 2 heads per core
ROPE_THETA = 10000.0
SQHD = math.sqrt(HD)
M_OFF = 18.0                      # softmax stability offset
NEG = -1.0e30

TCH = 512                         # t-chunk width in phase 1
NCH = ROWS // TCH                 # 8
CT = DM // 128                    # 16 contraction tiles
NT = T // 128                     # 16 key/query tiles per batch
IC = 512                          # query chunk in phase 2
NIC = T // IC                     # 4

F32 = mybir.dt.float32
F32R = mybir.dt.float32r
MULT = mybir.AluOpType.mult
ADD = mybir.AluOpType.add
EXP = mybir.ActivationFunctionType.Exp


def build_program(phases="123", loop_n=1):
    nc = bacc.Bacc("TRN2", target_bir_lowering=False, debug=False,
                   num_devices=NCORES)
    xT = nc.dram_tensor("xT", [DM, ROWS], F32, kind="ExternalInput").ap()
    wq = nc.dram_tensor("wq", [DM, HPC * HD], F32, kind="ExternalInput").ap()
    wk = nc.dram_tensor("wk", [DM, HPC * HD], F32, kind="ExternalInput").ap()
    wv = nc.dram_tensor("wv", [DM, HPC * HD], F32, kind="ExternalInput").ap()
    wp = nc.dram_tensor("wp", [HPC * HD, DM], F32, kind="ExternalInput").ap()
    cosT = nc.dram_tensor("cosT", [128, T], F32, kind="ExternalInput").ap()
    sinT = nc.dram_tensor("sinT", [128, T], F32, kind="ExternalInput").ap()
    prot = nc.dram_tensor("prot", [128, 128], F32, kind="ExternalInput").ap()
    biasd = nc.dram_tensor("biasd", [128, HPC, 16, 128], F32,
                           kind="ExternalInput").ap()
    ones128 = nc.dram_tensor("ones128", [128, 1], F32, kind="ExternalInput").ap()
    ones1 = nc.dram_tensor("ones1", [1, 128], F32, kind="ExternalInput").ap()
    out = nc.dram_tensor("out", [ROWS, DM], F32, kind="ExternalOutput").ap()

    xT3 = xT.rearrange("(o p) t -> p o t", p=128)

    with tile.TileContext(nc) as tc, ExitStack() as ctx:
        const = ctx.enter_context(tc.tile_pool(name="const", bufs=1))
        qkp = ctx.enter_context(tc.tile_pool(name="qk", bufs=1))

        q_sb = [qkp.tile([128, ROWS], F32R, tag=f"q{e}", name=f"q{e}")
                for e in range(HPC)]
        k_sb = [qkp.tile([128, ROWS], F32R, tag=f"k{e}", name=f"k{e}")
                for e in range(HPC)]
        v_keep = qkp.tile([128, B * NT, HPC * HD], F32R, tag="vk", name="vk")

        cos_sb = const.tile([128, T], F32, tag="cos")
        sin_sb = const.tile([128, T], F32, tag="sin")
        prot_sb = const.tile([128, 128], F32R, tag="prot")
        ones128_sb = const.tile([128, 1], F32R, tag="o128")
        ones1_sb = const.tile([1, 128], F32R, tag="o1")
        nc.sync.dma_start(cos_sb[:], cosT)
        nc.sync.dma_start(sin_sb[:], sinT)
        nc.sync.dma_start(prot_sb[:], prot.bitcast(F32R))
        nc.sync.dma_start(ones128_sb[:], ones128.bitcast(F32R))
        nc.sync.dma_start(ones1_sb[:], ones1.bitcast(F32R))

        if loop_n > 1:
            # timing mode: run the whole body loop_n times on-device
            ctx.enter_context(tc.For_i(0, loop_n, 1))

        # ---------------- phase 1: qkv + rope ----------------
        with tc.tile_pool(name="w1", bufs=1) as wpool, \
             tc.tile_pool(name="xt", bufs=17) as xpool, \
             tc.tile_pool(name="rope", bufs=3) as rpool, \
             tc.tile_pool(name="p1", bufs=4, space="PSUM") as ps1, \
             tc.tile_pool(name="pr", bufs=2, space="PSUM") as psr, \
             tc.tile_pool(name="pv", bufs=2, space="PSUM") as psv:
            wq_sb = wpool.tile([128, CT, HPC * HD], F32R, tag="wq")
            wk_sb = wpool.tile([128, CT, HPC * HD], F32R, tag="wk")
            wv_sb = wpool.tile([128, CT, HPC * HD], F32R, tag="wv")
            nc.sync.dma_start(wq_sb[:], wq.rearrange("(o p) e -> p o e", p=128).bitcast(F32R))
            nc.sync.dma_start(wk_sb[:], wk.rearrange("(o p) e -> p o e", p=128).bitcast(F32R))
            nc.sync.dma_start(wv_sb[:], wv.rearrange("(o p) e -> p o e", p=128).bitcast(F32R))

            for tchunk in range(NCH):
                t0 = tchunk * TCH
                xts = []
                for ci in range(CT):
                    xt_t = xpool.tile([128, TCH], F32R, tag="xt")
                    nc.sync.dma_start(xt_t[:], xT3[:, ci, t0:t0 + TCH].bitcast(F32R))
                    xts.append(xt_t)

                cs = slice(t0 % T, t0 % T + TCH)
                for dst, w_sb in ((q_sb, wq_sb), (k_sb, wk_sb)):
                    for et in range(HPC):
                        ps_q = ps1.tile([128, TCH], F32, tag="psq")
                        for ci in range(CT):
                            nc.tensor.matmul(ps_q[:],
                                             w_sb[:, ci, et * HD:(et + 1) * HD],
                                             xts[ci][:],
                                             start=(ci == 0), stop=(ci == CT - 1))
                        qraw = rpool.tile([128, TCH], F32R, tag="qraw")
                        nc.scalar.copy(qraw[:], ps_q[:])
                        ps_r = psr.tile([128, TCH], F32, tag="psrot")
                        nc.tensor.matmul(ps_r[:], prot_sb[:], qraw[:],
                                         start=True, stop=True)
                        tmp = rpool.tile([128, TCH], F32, tag="tmp")
                        nc.vector.tensor_mul(tmp[:], ps_r[:], sin_sb[:, cs])
                        dcols = dst[et][:, t0:t0 + TCH]
                        nc.vector.tensor_mul(dcols, qraw[:], cos_sb[:, cs])
                        nc.vector.tensor_add(dcols, dcols, tmp[:])

                for tt in range(TCH // 128):
                    ps_vt = psv.tile([128, HPC * HD], F32, tag="psv")
                    for ci in range(CT):
                        nc.tensor.matmul(ps_vt[:],
                                         xts[ci][:, tt * 128:(tt + 1) * 128],
                                         wv_sb[:, ci, :],
                                         start=(ci == 0), stop=(ci == CT - 1))
                    nc.scalar.copy(v_keep[:, t0 // 128 + tt, :], ps_vt[:])

        # ---------------- phase 2+3: attention + projection ----------------
        do2 = "2" in phases
        with tc.tile_pool(name="c2", bufs=1) as const2, \
             tc.tile_pool(name="wt", bufs=8) as wpool2, \
             tc.tile_pool(name="yb", bufs=2) as ypool, \
             tc.tile_pool(name="sm", bufs=2) as smpool, \
             tc.tile_pool(name="ost", bufs=3) as ostp, \
             tc.tile_pool(name="pssc", bufs=3, space="PSUM") as pssc, \
             tc.tile_pool(name="psacc", bufs=2, space="PSUM") as psacc, \
             tc.tile_pool(name="psm", bufs=1, space="PSUM") as psmisc, \
             tc.tile_pool(name="pso", bufs=2, space="PSUM") as pso:
            bias_sb = const2.tile([128, HPC, 16, 128], F32, tag="bias")
            nc.sync.dma_start(bias_sb[:], biasd)
            wp_sb = const2.tile([128, HPC, DM], F32R, tag="wp")
            nc.sync.dma_start(wp_sb[:], wp.rearrange("(o p) e -> p o e", p=128).bitcast(F32R))

            for b in range(B if do2 else 0):
                y_b = ypool.tile([128, HPC, T], F32R, tag="yb")
                for icx in range(NIC):
                    i0 = b * T + icx * IC
                    jt_hi = (icx + 1) * (IC // 128)
                    for hi in range(HPC):
                        ps_y = psacc.tile([128, IC], F32, tag="psy")
                        ps_misc = psmisc.tile([128, IC], F32, tag="psm")
                        ps_sum = ps_misc[0:1, :]
                        for jt in range(jt_hi):
                            # skip fully-masked query columns: i-tile >= jt
                            o = max(0, jt * 128 - icx * IC)
                            n = IC - o
                            ps_sc = pssc.tile([128, IC], F32, tag="pssc")
                            nc.tensor.matmul(
                                ps_sc[:, o:],
                                k_sb[hi][:, b * T + jt * 128: b * T + (jt + 1) * 128],
                                q_sb[hi][:, i0 + o:i0 + IC],
                                start=True, stop=True)
                            d0 = (icx * IC + o) // 128 - jt   # first delta >= 0
                            ps3 = ps_sc[:, o:].rearrange("p (a c) -> p a c", c=128)
                            nc.vector.scalar_tensor_tensor(
                                out=ps3, in0=ps3, scalar=1.0,
                                in1=bias_sb[:, hi, d0:d0 + n // 128, :],
                                op0=MULT, op1=ADD)
                            w_t = wpool2.tile([128, IC], F32R, tag="wt")
                            nc.scalar.activation(w_t[:, o:], ps_sc[:, o:], EXP,
                                                 bias=0.0, scale=1.0 / SQHD)
                            nc.tensor.matmul(ps_y[:, o:], v_keep[:, b * NT + jt, hi * HD:(hi + 1) * HD], w_t[:, o:],
                                             start=(jt == 0), stop=(jt == jt_hi - 1))
                            nc.tensor.matmul(ps_sum[:, o:], ones128_sb[:], w_t[:, o:],
                                             start=(jt == 0), stop=(jt == jt_hi - 1))
                        recip = smpool.tile([1, IC], F32R, tag="recip")
                        with nc.allow_low_precision(reason="f32r is 4-byte"):
                            nc.vector.reciprocal(recip[:], ps_sum)
                        # broadcast recip down 128 partitions (reuses misc bank)
                        nc.tensor.matmul(ps_misc[:], ones1_sb[:], recip[:],
                                         start=True, stop=True)
                        bca = smpool.tile([128, IC], F32, tag="bca")
                        nc.scalar.copy(bca[:], ps_misc[:])
                        nc.vector.tensor_mul(y_b[:, hi, icx * IC:(icx + 1) * IC],
                                             ps_y[:], bca[:])

                    if "3" not in phases:
                        continue
                    # projection of this i-chunk's rows (y ready for both heads)
                    for tt in range(icx * (IC // 128), (icx + 1) * (IC // 128)):
                        for ec in range(DM // 512):
                            ps_out = pso.tile([128, 512], F32, tag="pso")
                            for dt_ in range(HPC):
                                nc.tensor.matmul(ps_out[:],
                                                 y_b[:, dt_, tt * 128:(tt + 1) * 128],
                                                 wp_sb[:, dt_, ec * 512:(ec + 1) * 512],
                                                 start=(dt_ == 0), stop=(dt_ == HPC - 1))
                            o_stage = ostp.tile([128, 512], F32, tag="ost")
                            if (tt * 4 + ec) % 2 == 0:
                                nc.vector.tensor_copy(o_stage[:], ps_out[:])
                            else:
                                nc.scalar.copy(o_stage[:], ps_out[:])
                            r0 = b * T + tt * 128
                            nc.sync.dma_start(out[r0:r0 + 128, ec * 512:(ec + 1) * 512],
                                              o_stage[:])

    nc.compile()
    return nc


def _host_tensors():
    """Core-independent constant inputs."""
    hd2 = HD // 2
    inv_freq = 1.0 / (ROPE_THETA ** (np.arange(0, HD, 2, dtype=np.float64) / HD))
    ang = np.arange(T, dtype=np.float64)[:, None] * inv_freq[None, :]   # [T, 64]
    cos_h = np.cos(ang).T.astype(np.float32)                            # [64, T]
    sin_h = np.sin(ang).T.astype(np.float32)
    cosT = np.concatenate([cos_h, cos_h], axis=0)                       # [128, T]
    sinT = np.concatenate([sin_h, sin_h], axis=0)

    prot = np.zeros((128, 128), dtype=np.float32)
    for e in range(hd2):
        prot[e + hd2, e] = -1.0       # rot_e = -q_{e+64}  (e < 64)
        prot[e, e + hd2] = 1.0        # rot_e = +q_{e-64}  (e >= 64)

    ones128 = np.ones((128, 1), dtype=np.float32)
    ones1 = np.ones((1, 128), dtype=np.float32)
    return cosT, sinT, prot, ones128, ones1


def _bias_tiles(h0):
    """[128, HPC, 16, 128] additive pre-scale bias: sqrt(HD)*(alibi - M) or NEG."""
    jj = np.arange(128)[:, None]
    ii = np.arange(128)[None, :]
    rel = (jj - ii).astype(np.float64)          # (jj - ii)
    bias = np.empty((128, HPC, 16, 128), dtype=np.float32)
    for e in range(HPC):
        h = h0 + e
        slope = 2.0 ** (-8.0 * (h + 1) / H)
        for d in range(16):                      # d = it - jt >= 0
            v = SQHD * (slope * (rel - 128.0 * d) - M_OFF)
            tile_v = v.astype(np.float32)
            if d == 0:
                tile_v = np.where(jj > ii, NEG, tile_v)
            bias[:, e, d, :] = tile_v
    return bias


_NC_CACHE = {}


def _get_program():
    if "nc" not in _NC_CACHE:
        _NC_CACHE["nc"] = build_program()
    return _NC_CACHE["nc"]


def make_in_maps(x, W_qkv, W_proj):
    x = np.asarray(x, dtype=np.float32)
    W_qkv = np.asarray(W_qkv, dtype=np.float32)
    W_proj = np.asarray(W_proj, dtype=np.float32)

    xT = np.ascontiguousarray(x.reshape(ROWS, DM).T)        # [DM, ROWS]
    Wq, Wk, Wv = W_qkv[:, :DM], W_qkv[:, DM:2 * DM], W_qkv[:, 2 * DM:]
    cosT, sinT, prot, ones128, ones1 = _host_tensors()

    in_maps = []
    for c in range(NCORES):
        h0 = HPC * c
        cols = np.r_[h0 * HD:(h0 + 1) * HD, (h0 + 1) * HD:(h0 + 2) * HD]
        in_maps.append({
            "xT": xT,
            "wq": np.ascontiguousarray(Wq[:, cols]),
            "wk": np.ascontiguousarray(Wk[:, cols]),
            "wv": np.ascontiguousarray(Wv[:, cols]),
            "wp": np.ascontiguousarray(W_proj[cols, :]),
            "cosT": cosT,
            "sinT": sinT,
            "prot": prot,
            "biasd": _bias_tiles(h0),
            "ones128": ones128,
            "ones1": ones1,
        })
    return in_maps


def kernel(x, causal_mask, W_qkv, W_proj):
    del causal_mask  # always lower-triangular; causality is hardcoded
    nc = _get_program()
    in_maps = make_in_maps(x, W_qkv, W_proj)
    res = run_bass_kernel_spmd(nc, in_maps, core_ids=list(range(NCORES)))
    acc = np.zeros((ROWS, DM), dtype=np.float32)
    for c in range(NCORES):
        acc += res.results[c]["out"]
    return acc.reshape(B, T, DM)


# revision 20
# speedup vs baseline: 4.7152x; 1.0303x over previous
"""Trainium2 Bass kernel for CausalSelfAttention (RoPE + ALiBi + causal mask).

Sharding: 16 heads tensor-parallel across 8 NeuronCores (2 heads/core).
Per core:
  phase 1: qkv projection from replicated x^T; RoPE applied on the fly.
           q^T,k^T kept in SBUF [d, t]; v stored to DRAM in natural [t, d].
  phase 2: attention per (batch, head) in transposed layout
           S^T[j, i] = k^T.T @ q^T; ALiBi+mask via precomputed additive
           [128,128] delta tiles; exp on ScalarE; row-sums via ones-matmul;
           y^T accumulated on TensorE; normalized by broadcast reciprocal.
  phase 3: out partial = y @ W_proj (rows of the core's heads).
Host: sums the 8 partial outputs.

All matmuls run in float32r (TF32-like, full PE rate at free dim >= 256).
"""

import math
from contextlib import ExitStack

import numpy as np

import concourse.bass as bass
import concourse.mybir as mybir
import concourse.tile as tile
from concourse import bacc
from concourse.bass_utils import run_bass_kernel_spmd

B, T, DM = 2, 2048, 2048
H, HD = 16, 128
ROWS = B * T                      # 4096
NCORES = 8
HPC = H // NCORES                 # 2 heads per core
ROPE_THETA = 10000.0
SQHD = math.sqrt(HD)
M_OFF = 18.0                      # softmax stability offset
NEG = -1.0e30

TCH = 512                         # t-chunk width in phase 1
NCH = ROWS // TCH                 # 8
CT = DM // 128                    # 16 contraction tiles
NT = T // 128                     # 16 key/query tiles per batch
IC = 512                          # query chunk in phase 2
NIC = T // IC                     # 4

F32 = mybir.dt.float32
F32R = mybir.dt.float32r
MULT = mybir.AluOpType.mult
ADD = mybir.AluOpType.add
EXP = mybir.ActivationFunctionType.Exp


def build_program(phases="123", loop_n=1):
    nc = bacc.Bacc("TRN2", target_bir_lowering=False, debug=False,
                   num_devices=NCORES)
    xT = nc.dram_tensor("xT", [DM, ROWS], F32, kind="ExternalInput").ap()
    wq = nc.dram_tensor("wq", [DM, HPC * HD], F32, kind="ExternalInput").ap()
    wk = nc.dram_tensor("wk", [DM, HPC * HD], F32, kind="ExternalInput").ap()
    wv = nc.dram_tensor("wv", [DM, HPC * HD], F32, kind="ExternalInput").ap()
    wp = nc.dram_tensor("wp", [HPC * HD, DM], F32, kind="ExternalInput").ap()
    cosT = nc.dram_tensor("cosT", [128, T], F32, kind="ExternalInput").ap()
    sinT = nc.dram_tensor("sinT", [128, T], F32, kind="ExternalInput").ap()
    prot = nc.dram_tensor("prot", [128, 128], F32, kind="ExternalInput").ap()
    biasd = nc.dram_tensor("biasd", [128, HPC, 16, 128], F32,
                           kind="ExternalInput").ap()
    ones128 = nc.dram_tensor("ones128", [128, 1], F32, kind="ExternalInput").ap()
    ones1 = nc.dram_tensor("ones1", [1, 128], F32, kind="ExternalInput").ap()
    out = nc.dram_tensor("out", [ROWS, DM], F32, kind="ExternalOutput").ap()

    xT3 = xT.rearrange("(o p) t -> p o t", p=128)

    with tile.TileContext(nc) as tc, ExitStack() as ctx:
        const = ctx.enter_context(tc.tile_pool(name="const", bufs=1))
        qkp = ctx.enter_context(tc.tile_pool(name="qk", bufs=1))

        q_sb = [qkp.tile([128, ROWS], F32R, tag=f"q{e}", name=f"q{e}")
                for e in range(HPC)]
        k_sb = [qkp.tile([128, ROWS], F32R, tag=f"k{e}", name=f"k{e}")
                for e in range(HPC)]
        v_keep = qkp.tile([128, B * NT, HPC * HD], F32R, tag="vk", name="vk")

        cos_sb = const.tile([128, T], F32, tag="cos")
        sin_sb = const.tile([128, T], F32, tag="sin")
        prot_sb = const.tile([128, 128], F32R, tag="prot")
        ones128_sb = const.tile([128, 1], F32R, tag="o128")
        ones1_sb = const.tile([1, 128], F32R, tag="o1")
        nc.sync.dma_start(cos_sb[:], cosT)
        nc.sync.dma_start(sin_sb[:], sinT)
        nc.sync.dma_start(prot_sb[:], prot.bitcast(F32R))
        nc.sync.dma_start(ones128_sb[:], ones128.bitcast(F32R))
        nc.sync.dma_start(ones1_sb[:], ones1.bitcast(F32R))

        if loop_n > 1:
            # timing mode: run the whole body loop_n times on-device
            ctx.enter_context(tc.For_i(0, loop_n, 1))

        # ---------------- phase 1: qkv + rope ----------------
        with tc.tile_pool(name="w1", bufs=1) as wpool, \
             tc.tile_pool(name="xt", bufs=17) as xpool, \
             tc.tile_pool(name="rope", bufs=3) as rpool, \
             tc.tile_pool(name="p1", bufs=4, space="PSUM") as ps1, \
             tc.tile_pool(name="pr", bufs=2, space="PSUM") as psr, \
             tc.tile_pool(name="pv", bufs=2, space="PSUM") as psv:
            wq_sb = wpool.tile([128, CT, HPC * HD], F32R, tag="wq")
            wk_sb = wpool.tile([128, CT, HPC * HD], F32R, tag="wk")
            wv_sb = wpool.tile([128, CT, HPC * HD], F32R, tag="wv")
            nc.sync.dma_start(wq_sb[:], wq.rearrange("(o p) e -> p o e", p=128).bitcast(F32R))
            nc.sync.dma_start(wk_sb[:], wk.rearrange("(o p) e -> p o e", p=128).bitcast(F32R))
            nc.sync.dma_start(wv_sb[:], wv.rearrange("(o p) e -> p o e", p=128).bitcast(F32R))

            for tchunk in range(NCH):
                t0 = tchunk * TCH
                xts = []
                for ci in range(CT):
                    xt_t = xpool.tile([128, TCH], F32R, tag="xt")
                    nc.sync.dma_start(xt_t[:], xT3[:, ci, t0:t0 + TCH].bitcast(F32R))
                    xts.append(xt_t)

                cs = slice(t0 % T, t0 % T + TCH)
                for dst, w_sb in ((q_sb, wq_sb), (k_sb, wk_sb)):
                    for et in range(HPC):
                        ps_q = ps1.tile([128, TCH], F32, tag="psq")
                        for ci in range(CT):
                            nc.tensor.matmul(ps_q[:],
                                             w_sb[:, ci, et * HD:(et + 1) * HD],
                                             xts[ci][:],
                                             start=(ci == 0), stop=(ci == CT - 1))
                        qraw = rpool.tile([128, TCH], F32R, tag="qraw")
                        nc.scalar.copy(qraw[:], ps_q[:])
                        ps_r = psr.tile([128, TCH], F32, tag="psrot")
                        nc.tensor.matmul(ps_r[:], prot_sb[:], qraw[:],
                                         start=True, stop=True)
                        tmp = rpool.tile([128, TCH], F32, tag="tmp")
                        nc.vector.tensor_mul(tmp[:], ps_r[:], sin_sb[:, cs])
                        dcols = dst[et][:, t0:t0 + TCH]
                        nc.vector.tensor_mul(dcols, qraw[:], cos_sb[:, cs])
                        nc.vector.tensor_add(dcols, dcols, tmp[:])

                for tt in range(TCH // 128):
                    ps_vt = psv.tile([128, HPC * HD], F32, tag="psv")
                    for ci in range(CT):
                        nc.tensor.matmul(ps_vt[:],
                                         xts[ci][:, tt * 128:(tt + 1) * 128],
                                         wv_sb[:, ci, :],
                                         start=(ci == 0), stop=(ci == CT - 1))
                    nc.scalar.copy(v_keep[:, t0 // 128 + tt, :], ps_vt[:])

        # ---------------- phase 2+3: attention + projection ----------------
        do2 = "2" in phases
        with tc.tile_pool(name="c2", bufs=1) as const2, \
             tc.tile_pool(name="wt", bufs=8) as wpool2, \
             tc.tile_pool(name="yb", bufs=2) as ypool, \
             tc.tile_pool(name="sm", bufs=2) as smpool, \
             tc.tile_pool(name="ost", bufs=3) as ostp, \
             tc.tile_pool(name="pssc", bufs=3, space="PSUM") as pssc, \
             tc.tile_pool(name="psacc", bufs=2, space="PSUM") as psacc, \
             tc.tile_pool(name="psm", bufs=1, space="PSUM") as psmisc, \
             tc.tile_pool(name="pso", bufs=2, space="PSUM") as pso:
            bias_sb = const2.tile([128, HPC, 16, 128], F32, tag="bias")
            nc.sync.dma_start(bias_sb[:], biasd)
            wp_sb = const2.tile([128, HPC, DM], F32R, tag="wp")
            nc.sync.dma_start(wp_sb[:], wp.rearrange("(o p) e -> p o e", p=128).bitcast(F32R))

            for b in range(B if do2 else 0):
                y_b = ypool.tile([128, HPC, T], F32R, tag="yb")
                for icx in range(NIC):
                    i0 = b * T + icx * IC
                    jt_hi = (icx + 1) * (IC // 128)
                    for hi in range(HPC):
                        ps_y = psacc.tile([128, IC], F32, tag="psy")
                        ps_sum = psmisc.tile([1, IC], F32, tag="psm")
                        for jt in range(jt_hi):
                            # skip fully-masked query columns: i-tile >= jt
                            o = max(0, jt * 128 - icx * IC)
                            n = IC - o
                            ps_sc = pssc.tile([128, IC], F32, tag="pssc")
                            nc.tensor.matmul(
                                ps_sc[:, o:],
                                k_sb[hi][:, b * T + jt * 128: b * T + (jt + 1) * 128],
                                q_sb[hi][:, i0 + o:i0 + IC],
                                start=True, stop=True)
                            d0 = (icx * IC + o) // 128 - jt   # first delta >= 0
                            ps3 = ps_sc[:, o:].rearrange("p (a c) -> p a c", c=128)
                            nc.vector.scalar_tensor_tensor(
                                out=ps3, in0=ps3, scalar=1.0,
                                in1=bias_sb[:, hi, d0:d0 + n // 128, :],
                                op0=MULT, op1=ADD)
                            w_t = wpool2.tile([128, IC], F32R, tag="wt")
                            nc.scalar.activation(w_t[:, o:], ps_sc[:, o:], EXP,
                                                 bias=0.0, scale=1.0 / SQHD)
                            nc.tensor.matmul(ps_y[:, o:], v_keep[:, b * NT + jt, hi * HD:(hi + 1) * HD], w_t[:, o:],
                                             start=(jt == 0), stop=(jt == jt_hi - 1))
                            nc.tensor.matmul(ps_sum[0:1, o:], ones128_sb[:], w_t[:, o:],
                                             start=(jt == 0), stop=(jt == jt_hi - 1))
                        recip = smpool.tile([1, IC], F32R, tag="recip")
                        with nc.allow_low_precision(reason="f32r is 4-byte"):
                            nc.vector.reciprocal(recip[:], ps_sum[0:1, :])
                        # broadcast recip down 128 partitions; output goes to
                        # the scores pool so ps_sum's bank frees right after
                        # the reciprocal (avoids stalling next chunk's sums)
                        ps_b = pssc.tile([128, IC], F32, tag="pssc")
                        nc.tensor.matmul(ps_b[:], ones1_sb[:], recip[:],
                                         start=True, stop=True)
                        bca = smpool.tile([128, IC], F32, tag="bca")
                        nc.scalar.copy(bca[:], ps_b[:])
                        nc.vector.tensor_mul(y_b[:, hi, icx * IC:(icx + 1) * IC],
                                             ps_y[:], bca[:])

                    if "3" not in phases:
                        continue
                    # projection of this i-chunk's rows (y ready for both heads)
                    for tt in range(icx * (IC // 128), (icx + 1) * (IC // 128)):
                        for ec in range(DM // 512):
                            ps_out = pso.tile([128, 512], F32, tag="pso")
                            for dt_ in range(HPC):
                                nc.tensor.matmul(ps_out[:],
                                                 y_b[:, dt_, tt * 128:(tt + 1) * 128],
                                                 wp_sb[:, dt_, ec * 512:(ec + 1) * 512],
                                                 start=(dt_ == 0), stop=(dt_ == HPC - 1))
                            o_stage = ostp.tile([128, 512], F32, tag="ost")
                            if (tt * 4 + ec) % 2 == 0:
                                nc.vector.tensor_copy(o_stage[:], ps_out[:])
                            else:
                                nc.scalar.copy(o_stage[:], ps_out[:])
                            r0 = b * T + tt * 128
                            nc.sync.dma_start(out[r0:r0 + 128, ec * 512:(ec + 1) * 512],
                                              o_stage[:])

    nc.compile()
    return nc


def _host_tensors():
    """Core-independent constant inputs."""
    hd2 = HD // 2
    inv_freq = 1.0 / (ROPE_THETA ** (np.arange(0, HD, 2, dtype=np.float64) / HD))
    ang = np.arange(T, dtype=np.float64)[:, None] * inv_freq[None, :]   # [T, 64]
    cos_h = np.cos(ang).T.astype(np.float32)                            # [64, T]
    sin_h = np.sin(ang).T.astype(np.float32)
    cosT = np.concatenate([cos_h, cos_h], axis=0)                       # [128, T]
    sinT = np.concatenate([sin_h, sin_h], axis=0)

    prot = np.zeros((128, 128), dtype=np.float32)
    for e in range(hd2):
        prot[e + hd2, e] = -1.0       # rot_e = -q_{e+64}  (e < 64)
        prot[e, e + hd2] = 1.0        # rot_e = +q_{e-64}  (e >= 64)

    ones128 = np.ones((128, 1), dtype=np.float32)
    ones1 = np.ones((1, 128), dtype=np.float32)
    return cosT, sinT, prot, ones128, ones1


def _bias_tiles(h0):
    """[128, HPC, 16, 128] additive pre-scale bias: sqrt(HD)*(alibi - M) or NEG."""
    jj = np.arange(128)[:, None]
    ii = np.arange(128)[None, :]
    rel = (jj - ii).astype(np.float64)          # (jj - ii)
    bias = np.empty((128, HPC, 16, 128), dtype=np.float32)
    for e in range(HPC):
        h = h0 + e
        slope = 2.0 ** (-8.0 * (h + 1) / H)
        for d in range(16):                      # d = it - jt >= 0
            v = SQHD * (slope * (rel - 128.0 * d) - M_OFF)
            tile_v = v.astype(np.float32)
            if d == 0:
                tile_v = np.where(jj > ii, NEG, tile_v)
            bias[:, e, d, :] = tile_v
    return bias


_NC_CACHE = {}


def _get_program():
    if "nc" not in _NC_CACHE:
        _NC_CACHE["nc"] = build_program()
    return _NC_CACHE["nc"]


def make_in_maps(x, W_qkv, W_proj):
    x = np.asarray(x, dtype=np.float32)
    W_qkv = np.asarray(W_qkv, dtype=np.float32)
    W_proj = np.asarray(W_proj, dtype=np.float32)

    xT = np.ascontiguousarray(x.reshape(ROWS, DM).T)        # [DM, ROWS]
    Wq, Wk, Wv = W_qkv[:, :DM], W_qkv[:, DM:2 * DM], W_qkv[:, 2 * DM:]
    cosT, sinT, prot, ones128, ones1 = _host_tensors()

    in_maps = []
    for c in range(NCORES):
        h0 = HPC * c
        cols = np.r_[h0 * HD:(h0 + 1) * HD, (h0 + 1) * HD:(h0 + 2) * HD]
        in_maps.append({
            "xT": xT,
            "wq": np.ascontiguousarray(Wq[:, cols]),
            "wk": np.ascontiguousarray(Wk[:, cols]),
            "wv": np.ascontiguousarray(Wv[:, cols]),
            "wp": np.ascontiguousarray(W_proj[cols, :]),
            "cosT": cosT,
            "sinT": sinT,
            "prot": prot,
            "biasd": _bias_tiles(h0),
            "ones128": ones128,
            "ones1": ones1,
        })
    return in_maps


def kernel(x, causal_mask, W_qkv, W_proj):
    del causal_mask  # always lower-triangular; causality is hardcoded
    nc = _get_program()
    in_maps = make_in_maps(x, W_qkv, W_proj)
    res = run_bass_kernel_spmd(nc, in_maps, core_ids=list(range(NCORES)))
    acc = np.zeros((ROWS, DM), dtype=np.float32)
    for c in range(NCORES):
        acc += res.results[c]["out"]
    return acc.reshape(B, T, DM)
